# revision 73
# baseline (speedup 1.0000x reference)
"""AttentionGNNLayer Trainium2 kernel (8 NeuronCores, SPMD).

Math:  out = relu(segment_sum(h_proj[senders] * a[senders, receivers][:, None],
                              receivers, N))
with h_proj = h @ W, a = (h@Wq + bq) @ (h@Wk + bk)^T.

Sharding strategy: shard RECEIVER nodes across the 8 cores (1024 nodes each).
The edge list enters the kernel only through a per-core count matrix
Ct_c[m, n_loc] = #edges (m -> n_loc + 1024*c), built host-side while sharding
(pure index preprocessing). Per core, with n restricted to its 1024-node slice:

    G    = (Wq Wk^T) @ h_loc^T            (256 x 1024)   tiny
    A    = h @ G   (== q @ k_c^T)         (8192 x 1024)
    S    = Ct_c * A                       (8192 x 1024)  sparse-masked logits
    P    = S^T @ h                        (1024 x 256)
    out_c = relu((P @ W)^T)               (256 x 1024)   == relu(S^T @ h_proj)^T

The two O(N*NL*D) matmuls (A and P) are the irreducible compute; everything
else is O(D^2*NL). All in bf16 with f32 PSUM accumulation; no collectives.
bq and bk are asserted zero (the module spec fills both with zeros).
Output is produced transposed and un-transposed on the host.

Schedule: the 1024 local receiver columns are processed in TWO passes of 512
columns; each pass streams the 64 sender chunks (A -> S -> P accumulate).
Pass 0's epilogue (PSUM copy, W-apply matmuls, relu, output DMA) and the
second column block of G are interleaved into the following pass's matmul
stream, so only pass 1's 512-column epilogue sits on the critical tail.
A and P matmuls are software-pipelined (A[c+1] issued before P[c]) so the
tensor engine never waits on the vector engine's S = Ct*A.

DMA: all queues share ~358 GB/s of HBM bandwidth.  The latency-critical
stream (SM piece gating G, small first-pieces of hT/Ct/hN covering the
first chunks, then full hT/hN pieces in need-time order) lives alone on the
sync queue where in-order draining gives strict priority and all its
triggers precede the in-loop Ct stream in program order (completion
semaphores are recycled globally; this ordering keeps reuse waits dead).
Ct streams on the scalar queue backpressured to 4 tiles of lookahead, with
each pass's first tiles pre-triggered from the previous pass; outputs go on
the gpsimd queue.
"""

import sys

sys.path.insert(0, "/opt/trn_rl_repo")
sys.path.insert(0, "/opt/pypackages")

import numpy as np
import ml_dtypes

N_NODES = 8192
D = 256
N_CORES = 8
NL = N_NODES // N_CORES  # 1024 receiver nodes per core
NCHUNK = N_NODES // 128  # 64 m-chunks of 128 rows
NB = 2  # column-block passes of 512 receiver columns
BW = NL // NB  # 512

BF16 = ml_dtypes.bfloat16

_graph_cache = {}


def _build_graph():
    import concourse.bacc as bacc
    import concourse.mybir as mybir
    import concourse.tile as tile

    fp32 = mybir.dt.float32
    bf16 = mybir.dt.bfloat16
    uint8 = mybir.dt.uint8

    nc = bacc.Bacc("TRN2", target_bir_lowering=False, debug=False)

    # h^T pieces: piece k>=1 = m-chunks 8k..8k+7; chunk c at cols
    # ft*1024 + (c%8)*128 for f-half ft.  Piece 0 is chunk-major
    # (cols c*256 + ft*128) so its first half covers chunks 0-3 and can be
    # DMA'd separately - no redundant "first-piece" copies needed.
    hT_d = nc.declare_dram_parameter("hT", [8, 128, 2048], bf16, isOutput=False)
    # h row pieces: piece g = chunks 8g..8g+7, chunk c at cols (c%8)*256
    hN_d = nc.declare_dram_parameter("hN", [8, 128, 2048], bf16, isOutput=False)
    # unpacked counts for block-0 chunks 0-3 (chunk c at cols c*512): lets
    # S[0..3] start before Ct quad 1 lands; quad 0 is then only needed for
    # pass 1
    ctf_d = nc.declare_dram_parameter("ctf", [128, 2048], uint8, isOutput=False)
    # params packed into two single-DMA pieces (fewer completions = less
    # early-latency jitter): SMA[.., ft*768+] = [WqkT (256) | hTloc blk0
    # (512)] gates G block 0; SMB[.., ft*768+] = [hTloc blk1 (512) | W (256)]
    sma_d = nc.declare_dram_parameter("SMA", [128, 1536], bf16, isOutput=False)
    smb_d = nc.declare_dram_parameter("SMB", [128, 1536], bf16, isOutput=False)
    # counts: [block][quad j][128][chunk 4j+i at cols i*512]
    ct_d = nc.declare_dram_parameter("Ct", [NB, NCHUNK // 4, 128, 2048], uint8, isOutput=False)
    # out[blk*2+dh][p][n] = relu(agg)[dh*128+p, blk*512+n]
    out_d = nc.declare_dram_parameter("out", [2 * NB, 128, BW], fp32, isOutput=True)

    Relu = mybir.ActivationFunctionType.Relu
    Copy = mybir.ActivationFunctionType.Copy

    with tile.TileContext(nc) as tc:
        with (
            tc.tile_pool(name="big", bufs=1) as big,
            tc.tile_pool(name="ct", bufs=5) as ctp,
            tc.tile_pool(name="s", bufs=5) as sp,
            tc.tile_pool(name="apsum", bufs=3, space="PSUM") as apsum,
            tc.tile_pool(name="ptpsum", bufs=1, space="PSUM") as ptpsum,
            tc.tile_pool(name="gpsum", bufs=2, space="PSUM") as gpsum,
        ):
            # ---- critical input stream on the sync queue, in need-time
            # order ----
            SMA = big.tile([128, 1536], bf16, tag="SMA", name="SMA")
            SMB = big.tile([128, 1536], bf16, tag="SMB", name="SMB")
            hT = [
                big.tile([128, 2048], bf16, tag=f"hT{q}", name=f"hT{q}")
                for q in range(8)
            ]
            hN = [
                big.tile([128, 2048], bf16, tag=f"hN{g}", name=f"hN{g}")
                for g in range(8)
            ]
            ctf = big.tile([128, 2048], uint8, tag="ctf", name="ctf")
            # pad so the ct pool's tiles land 512B-aligned in SBUF (the DVE
            # S-mult measurably slows on misaligned uint8 operands)
            big.tile([128, 384], uint8, tag="pad", name="pad")
            ct_pref = {}
            # a single queue tops out ~304 GB/s of the ~358 aggregate, so
            # the second-priority early pieces ride the (otherwise idle)
            # gpsimd queue in parallel with the critical sync stream
            nc.sync.dma_start(SMA[:], sma_d[:])
            nc.sync.dma_start(hT[0][:, 0:1024], hT_d[0, :, 0:1024])
            nc.sync.dma_start(ctf[:], ctf_d[:])
            nc.sync.dma_start(hN[0][:, 0:1024], hN_d[0, :, 0:1024])
            nc.gpsimd.dma_start(hT[0][:, 1024:2048], hT_d[0, :, 1024:2048])
            nc.gpsimd.dma_start(hN[0][:, 1024:2048], hN_d[0, :, 1024:2048])
            nc.gpsimd.dma_start(hT[1][:, 0:1024], hT_d[1, :, 0:1024])
            nc.gpsimd.dma_start(hN[1][:, 0:1024], hN_d[1, :, 0:1024])
            nc.gpsimd.dma_start(hT[1][:, 1024:2048], hT_d[1, :, 1024:2048])
            nc.gpsimd.dma_start(hN[1][:, 1024:2048], hN_d[1, :, 1024:2048])
            nc.sync.dma_start(SMB[:], smb_d[:])
            for q in range(2, 8):
                nc.sync.dma_start(hT[q][:], hT_d[q])
                nc.sync.dma_start(hN[q][:], hN_d[q])

            # ---- PE warm-up: keep the HAM activity window busy during the
            # initial DMA wait so real matmuls start at 2.4 GHz.  memset on
            # gpsimd (starts earliest); psum from the apsum ring so G's
            # gpsum tiles are not blocked behind warm-up retirement ----
            wsrc = big.tile([128, 512], bf16, tag="wsrc", name="wsrc")
            nc.gpsimd.memset(wsrc[:], 0.0)
            for wi in range(7):
                wps = apsum.tile([128, 512], fp32, tag="a")
                nc.tensor.matmul(
                    wps[:], wsrc[:, :128], wsrc[:], start=True, stop=True
                )

            # pre-issue Ct quad 1 so the scalar queue's first in-loop
            # trigger is not serialized behind the G cast
            ctt1 = ctp.tile([128, 2048], uint8, tag="ct")
            nc.scalar.dma_start(ctt1[:], ct_d[0, 1])
            ct_pref[(0, 1)] = ctt1

            def fetch_ct(b, j):
                if (b, j) in ct_pref:
                    return ct_pref.pop((b, j))
                t = ctp.tile([128, 2048], uint8, tag="ct")
                nc.scalar.dma_start(t[:], ct_d[b, j])
                return t

            # ---- G = (Wq Wk^T) @ h_loc^T -> [2][128 f, NL]; bk == 0 so no
            # bias.  Block nk is consumed by pass nk; block 1 is emitted
            # inside pass 0's stream once SM piece b has landed ----
            G = [big.tile([128, NL], bf16, tag=f"G{t}", name=f"G{t}") for t in range(2)]

            def make_g(nk, gf):
                def emit():
                    ps = gpsum.tile([128, BW], fp32, tag="g")
                    for ft in range(2):
                        mov = (
                            SMA[:, ft * 768 + D : ft * 768 + D + BW]
                            if nk == 0
                            else SMB[:, ft * 768 : ft * 768 + BW]
                        )
                        nc.tensor.matmul(
                            ps[:],
                            SMA[:, ft * 768 + gf * 128 : ft * 768 + (gf + 1) * 128],
                            mov,
                            start=(ft == 0),
                            stop=(ft == 1),
                        )
                    if gf == 0:
                        nc.scalar.activation(
                            G[gf][:, nk * BW : (nk + 1) * BW], ps[:], Copy
                        )
                    else:
                        nc.vector.tensor_copy(
                            G[gf][:, nk * BW : (nk + 1) * BW], ps[:]
                        )
                return emit

            make_g(0, 0)()
            make_g(0, 1)()
            # two filler matmuls keep the PE busy through the G-cast
            # window so the clock ramp is not reset before A[0]
            for wi in range(2):
                wps = apsum.tile([128, 512], fp32, tag="a")
                nc.tensor.matmul(
                    wps[:], wsrc[:, :128], wsrc[:], start=True, stop=True
                )

            # ---- two column-block passes over all 64 m-chunks ----
            # deferred tensor-engine work dropped into the next pass's stream
            # (G block 1 at c=20/21: SM piece b lands behind hN1 on the queue)
            deferred = {20: make_g(1, 0), 21: make_g(1, 1)}
            for blk in range(NB):
                PT = [
                    ptpsum.tile([128, BW], fp32, tag=f"pt{fh}", name=f"PT{blk}{fh}")
                    for fh in range(2)
                ]
                sts = {}
                for c in range(NCHUNK + 3):
                    if c < NCHUNK:
                        # quad 0 covers chunks 0-3, served by ctf in pass 0
                        if c % 4 == 0 and not (blk == 0 and c == 0):
                            ctt = fetch_ct(blk, c // 4)
                        # A[c] = h-chunk @ G-block  (2 f-half matmuls);
                        # piece 0 is chunk-major (see hT_d comment)
                        aps = apsum.tile([128, BW], fp32, tag="a")
                        for ft in range(2):
                            stat = (
                                hT[c // 8][:, (c % 8) * 256 + ft * 128 : (c % 8) * 256 + (ft + 1) * 128]
                                if c < 16
                                else hT[c // 8][:, ft * 1024 + (c % 8) * 128 : ft * 1024 + (c % 8 + 1) * 128]
                            )
                            nc.tensor.matmul(
                                aps[:],
                                stat,
                                G[ft][:, blk * BW : (blk + 1) * BW],
                                start=(ft == 0),
                                stop=(ft == 1),
                            )
                        # S[c] = Ct * A[c]  (vector engine, PSUM -> SBUF bf16)
                        st = sp.tile([128, BW], bf16, tag="s")
                        csrc = (
                            ctf[:, c * 512 : (c + 1) * 512]
                            if blk == 0 and c < 4
                            else ctt[:, (c % 4) * 512 : (c % 4) * 512 + BW]
                        )
                        nc.vector.tensor_mul(st[:], aps[:], csrc)
                    # P[c-3]: PT[fh] += hN-chunk^T @ S[c-3]  (issued three
                    # chunks late: ~1.8us of slack absorbs the periodic
                    # ~0.85us external hiccup that stalls the vector engine)
                    if c >= 3:
                        cp = c - 3
                        st_prev = sts.pop(cp)
                        for fh in range(2):
                            pstat = (
                                hN[cp // 8][:, (cp % 8) * 256 + fh * 128 : (cp % 8) * 256 + (fh + 1) * 128]
                            )
                            nc.tensor.matmul(
                                PT[fh][:],
                                pstat,
                                st_prev[:],
                                start=(cp == 0),
                                stop=(cp == NCHUNK - 1),
                            )
                    if c < NCHUNK:
                        sts[c] = st
                    if c in deferred:
                        deferred.pop(c)()
                    # pre-trigger the next pass's first Ct quads so they are
                    # not serialized behind the pass-boundary PSUM casts on
                    # the scalar queue
                    if blk == 0 and c == 56:
                        ct_pref[(1, 0)] = fetch_ct(1, 0)
                    if blk == 0 and c == 60:
                        ct_pref[(1, 1)] = fetch_ct(1, 1)

                # ---- pass epilogue: PT -> SBUF bf16 now; agg = W^T @ PT,
                # relu and output DMA deferred into the next pass's stream
                # (for the last pass: emitted immediately, split across
                # scalar/vector for the shortest tail) ----
                last = blk == NB - 1
                PTs = [
                    big.tile([128, BW], bf16, tag=f"PTs{blk}{fh}", name=f"PTs{blk}{fh}")
                    for fh in range(2)
                ]
                if last:
                    nc.scalar.activation(PTs[0][:], PT[0][:], Copy)
                    nc.vector.tensor_copy(PTs[1][:], PT[1][:])
                else:
                    # keep the vector engine free for the next pass's S-mults
                    nc.scalar.activation(PTs[0][:], PT[0][:], Copy)
                    nc.scalar.activation(PTs[1][:], PT[1][:], Copy)
                outS = big.tile([128, NL], fp32, tag=f"out{blk}", name=f"out{blk}")

                def make_agg(blk, dh, PTs=PTs, outS=outS):
                    def emit():
                        agg = gpsum.tile([128, BW], fp32, tag="g")
                        for ft in range(2):
                            nc.tensor.matmul(
                                agg[:],
                                SMB[:, ft * 768 + BW + dh * 128 : ft * 768 + BW + (dh + 1) * 128],
                                PTs[ft][:],
                                start=(ft == 0),
                                stop=(ft == 1),
                            )
                        if dh == 0:
                            nc.scalar.activation(
                                outS[:, dh * BW : (dh + 1) * BW], agg[:], Relu
                            )
                        else:
                            nc.vector.tensor_scalar_max(
                                outS[:, dh * BW : (dh + 1) * BW], agg[:], 0.0
                            )
                        eng = nc.scalar if (blk == NB - 1 and dh == 0) else nc.gpsimd
                        eng.dma_start(
                            out_d[blk * 2 + dh], outS[:, dh * BW : (dh + 1) * BW]
                        )
                    return emit

                if last:
                    make_agg(blk, 0)()
                    make_agg(blk, 1)()
                else:
                    deferred = {2: make_agg(blk, 0), 3: make_agg(blk, 1)}

    nc.compile()
    return nc


def _get_graph():
    if "nc" not in _graph_cache:
        _graph_cache["nc"] = _build_graph()
    return _graph_cache["nc"]


def make_in_maps(h, W, Wq, bq, Wk, bk, senders, receivers):
    h = np.asarray(h, dtype=np.float32)
    W = np.asarray(W, dtype=np.float32)
    Wq = np.asarray(Wq, dtype=np.float32)
    Wk = np.asarray(Wk, dtype=np.float32)
    bq = np.asarray(bq, dtype=np.float32)
    bk = np.asarray(bk, dtype=np.float32)
    s = np.asarray(senders).astype(np.int64)
    r = np.asarray(receivers).astype(np.int64)

    # bq == bk == 0 (module spec fills both with zeros) lets
    # A = h @ ((Wq Wk^T) h_loc^T) stand in exactly for q @ k^T.
    assert not np.any(bq), "kernel fast path assumes bq == 0"
    assert not np.any(bk), "kernel fast path assumes bk == 0"

    hT = np.ascontiguousarray(h.T).astype(BF16)  # [256, 8192]
    # piece k = m-chunks 8k..8k+7: cols [f-half0 1024 | f-half1 1024];
    # piece 0 is chunk-major (cols c*256 + ft*128) so its halves split at
    # chunk 4 for the early DMA
    hT8 = (
        hT.reshape(2, 128, 8, 1024)
        .transpose(2, 1, 0, 3)
        .reshape(8, 128, 2048)
    )
    hT8 = np.ascontiguousarray(hT8)
    for k in range(2):
        hT8[k] = (
            hT8[k].reshape(128, 2, 8, 128).transpose(0, 2, 1, 3).reshape(128, 2048)
        )
    hN8 = (
        h.astype(BF16)
        .reshape(8, 8, 128, D)
        .transpose(0, 2, 1, 3)
        .reshape(8, 128, 8 * D)
    )
    # folded attention weight product (parameter preprocessing):
    # G = (Wq Wk^T) h_loc^T  ==  q-free form of q @ k_c^T
    WqkT = (Wk @ Wq.T).astype(BF16).reshape(2, 128, D)
    Wb = W.astype(BF16).reshape(2, 128, D)

    in_maps = []
    for c in range(N_CORES):
        lo = c * NL
        m = (r >= lo) & (r < lo + NL)
        idx = s[m] * NL + (r[m] - lo)
        Ct = np.bincount(idx, minlength=N_NODES * NL)
        assert Ct.max() < 128
        Ct = Ct.astype(np.uint8).reshape(NCHUNK, 128, NL)
        # [blk][quad j][128][chunk 4j+i at cols i*512]
        Ctb = np.empty((NB, NCHUNK // 4, 128, 2048), np.uint8)
        for b in range(NB):
            blkslice = Ct[:, :, b * BW : (b + 1) * BW]  # [64,128,512]
            for i in range(4):
                Ctb[b, :, :, i * 512 : (i + 1) * 512] = blkslice[i::4]
        hTloc = hT.reshape(2, 128, N_NODES)[:, :, lo : lo + NL]
        # SMA = per f-half [WqkT | hTloc blk0]; SMB = [hTloc blk1 | W]
        SMA = np.concatenate(
            [np.concatenate([WqkT[t], hTloc[t][:, 0:BW]], axis=1) for t in range(2)],
            axis=1,
        )
        SMB = np.concatenate(
            [np.concatenate([hTloc[t][:, BW:NL], Wb[t]], axis=1) for t in range(2)],
            axis=1,
        )
        # unpacked block-0 counts for chunks 0-3 at cols c*512
        ctf = np.ascontiguousarray(
            np.concatenate([Ct[cc, :, 0:BW] for cc in range(4)], axis=1)
        )
        in_maps.append(
            {
                "hT": hT8,
                "hN": hN8,
                "SMA": np.ascontiguousarray(SMA),
                "SMB": np.ascontiguousarray(SMB),
                "Ct": Ctb,
                "ctf": ctf,
            }
        )
    return in_maps


def assemble_output(results):
    out = np.empty((N_NODES, D), np.float32)
    for c in range(N_CORES):
        res = np.asarray(results[c]["out"]).reshape(NB, 2, 128, BW)
        for b in range(NB):
            # res[b][dh, p, n] -> out[lo + b*512 + n, dh*128 + p]
            out[c * NL + b * BW : c * NL + (b + 1) * BW] = (
                res[b].transpose(2, 0, 1).reshape(BW, D)
            )
    return out


def kernel(h, W, Wq, bq, Wk, bk, senders, receivers):
    from concourse.bass_utils import run_bass_kernel_spmd

    in_maps = make_in_maps(h, W, Wq, bq, Wk, bk, senders, receivers)
    nc = _get_graph()
    res = run_bass_kernel_spmd(nc, in_maps, list(range(N_CORES))).results
    return assemble_output(res)


# revision 74
# speedup vs baseline: 1.0733x; 1.0733x over previous
"""AttentionGNNLayer Trainium2 kernel (8 NeuronCores, SPMD).

Math:  out = relu(segment_sum(h_proj[senders] * a[senders, receivers][:, None],
                              receivers, N))
with h_proj = h @ W, a = (h@Wq + bq) @ (h@Wk + bk)^T.

Sharding strategy: shard RECEIVER nodes across the 8 cores (1024 nodes each).
The edge list enters the kernel only through a per-core count matrix
Ct_c[m, n_loc] = #edges (m -> n_loc + 1024*c), built host-side while sharding
(pure index preprocessing). Per core, with n restricted to its 1024-node slice:

    G    = (Wq Wk^T) @ h_loc^T            (256 x 1024)   tiny
    A    = h @ G   (== q @ k_c^T)         (8192 x 1024)
    S    = Ct_c * A                       (8192 x 1024)  sparse-masked logits
    P    = S^T @ h                        (1024 x 256)
    out_c = relu((P @ W)^T)               (256 x 1024)   == relu(S^T @ h_proj)^T

The two O(N*NL*D) matmuls (A and P) are the irreducible compute; everything
else is O(D^2*NL). All in bf16 with f32 PSUM accumulation; no collectives.
bq and bk are asserted zero (the module spec fills both with zeros).
Output is produced transposed and un-transposed on the host.

Schedule: the 1024 local receiver columns are processed in TWO passes of 512
columns; each pass streams the 64 sender chunks (A -> S -> P accumulate).
Pass 0's epilogue (PSUM copy, W-apply matmuls, relu, output DMA) and the
second column block of G are interleaved into the following pass's matmul
stream, so only pass 1's 512-column epilogue sits on the critical tail.
A and P matmuls are software-pipelined (A[c+1] issued before P[c]) so the
tensor engine never waits on the vector engine's S = Ct*A.

DMA: all queues share ~358 GB/s of HBM bandwidth.  The latency-critical
stream (SM piece gating G, small first-pieces of hT/Ct/hN covering the
first chunks, then full hT/hN pieces in need-time order) lives alone on the
sync queue where in-order draining gives strict priority and all its
triggers precede the in-loop Ct stream in program order (completion
semaphores are recycled globally; this ordering keeps reuse waits dead).
Ct streams on the scalar queue backpressured to 4 tiles of lookahead, with
each pass's first tiles pre-triggered from the previous pass; outputs go on
the gpsimd queue.
"""

import sys

sys.path.insert(0, "/opt/trn_rl_repo")
sys.path.insert(0, "/opt/pypackages")

import numpy as np
import ml_dtypes

N_NODES = 8192
D = 256
N_CORES = 8
NL = N_NODES // N_CORES  # 1024 receiver nodes per core
NCHUNK = N_NODES // 128  # 64 m-chunks of 128 rows
NB = 2  # column-block passes of 512 receiver columns
BW = NL // NB  # 512

BF16 = ml_dtypes.bfloat16

_graph_cache = {}


def _build_graph():
    import concourse.bacc as bacc
    import concourse.mybir as mybir
    import concourse.tile as tile

    fp32 = mybir.dt.float32
    bf16 = mybir.dt.bfloat16
    uint8 = mybir.dt.uint8

    nc = bacc.Bacc("TRN2", target_bir_lowering=False, debug=False)

    # h^T pieces: piece k>=1 = m-chunks 8k..8k+7; chunk c at cols
    # ft*1024 + (c%8)*128 for f-half ft.  Piece 0 is chunk-major
    # (cols c*256 + ft*128) so its first half covers chunks 0-3 and can be
    # DMA'd separately - no redundant "first-piece" copies needed.
    hT_d = nc.declare_dram_parameter("hT", [8, 128, 2048], bf16, isOutput=False)
    # h row pieces: piece g = chunks 8g..8g+7, chunk c at cols (c%8)*256
    hN_d = nc.declare_dram_parameter("hN", [8, 128, 2048], bf16, isOutput=False)
    # unpacked counts for block-0 chunks 0-3 (chunk c at cols c*512): lets
    # S[0..3] start before Ct quad 1 lands; quad 0 is then only needed for
    # pass 1
    ctf_d = nc.declare_dram_parameter("ctf", [128, 2048], uint8, isOutput=False)
    # params packed into two single-DMA pieces (fewer completions = less
    # early-latency jitter): SMA[.., ft*768+] = [WqkT (256) | hTloc blk0
    # (512)] gates G block 0; SMB[.., ft*768+] = [hTloc blk1 (512) | W (256)]
    sma_d = nc.declare_dram_parameter("SMA", [128, 1536], bf16, isOutput=False)
    smb_d = nc.declare_dram_parameter("SMB", [128, 1536], bf16, isOutput=False)
    # counts: [block][quad j][128][chunk 4j+i at cols i*512]
    ct_d = nc.declare_dram_parameter("Ct", [NB, NCHUNK // 4, 128, 2048], uint8, isOutput=False)
    # out[blk*2+dh][p][n] = relu(agg)[dh*128+p, blk*512+n]
    out_d = nc.declare_dram_parameter("out", [2 * NB, 128, BW], fp32, isOutput=True)

    Relu = mybir.ActivationFunctionType.Relu
    Copy = mybir.ActivationFunctionType.Copy

    with tile.TileContext(nc) as tc:
        with (
            tc.tile_pool(name="big", bufs=1) as big,
            tc.tile_pool(name="ct", bufs=5) as ctp,
            tc.tile_pool(name="s", bufs=5) as sp,
            tc.tile_pool(name="apsum", bufs=3, space="PSUM") as apsum,
            tc.tile_pool(name="ptpsum", bufs=1, space="PSUM") as ptpsum,
            tc.tile_pool(name="gpsum", bufs=2, space="PSUM") as gpsum,
        ):
            # ---- critical input stream on the sync queue, in need-time
            # order ----
            SMA = big.tile([128, 1536], bf16, tag="SMA", name="SMA")
            SMB = big.tile([128, 1536], bf16, tag="SMB", name="SMB")
            hT = [
                big.tile([128, 2048], bf16, tag=f"hT{q}", name=f"hT{q}")
                for q in range(8)
            ]
            hN = [
                big.tile([128, 2048], bf16, tag=f"hN{g}", name=f"hN{g}")
                for g in range(8)
            ]
            ctf = big.tile([128, 2048], uint8, tag="ctf", name="ctf")
            # pad so the ct pool's tiles land 512B-aligned in SBUF (the DVE
            # S-mult measurably slows on misaligned uint8 operands)
            big.tile([128, 384], uint8, tag="pad", name="pad")
            ct_pref = {}
            nc.sync.dma_start(SMA[:], sma_d[:])
            # pieces 0-1 are chunk-major and land in 4-chunk halves so
            # early supply tracks demand finely
            nc.sync.dma_start(hT[0][:, 0:1024], hT_d[0, :, 0:1024])
            nc.sync.dma_start(ctf[:], ctf_d[:])
            nc.sync.dma_start(hN[0][:, 0:1024], hN_d[0, :, 0:1024])
            nc.sync.dma_start(hT[0][:, 1024:2048], hT_d[0, :, 1024:2048])
            nc.sync.dma_start(hN[0][:, 1024:2048], hN_d[0, :, 1024:2048])
            nc.sync.dma_start(hT[1][:, 0:1024], hT_d[1, :, 0:1024])
            nc.sync.dma_start(hN[1][:, 0:1024], hN_d[1, :, 0:1024])
            nc.sync.dma_start(hT[1][:, 1024:2048], hT_d[1, :, 1024:2048])
            nc.sync.dma_start(hN[1][:, 1024:2048], hN_d[1, :, 1024:2048])
            nc.sync.dma_start(SMB[:], smb_d[:])
            for q in range(2, 8):
                nc.sync.dma_start(hT[q][:], hT_d[q])
                nc.sync.dma_start(hN[q][:], hN_d[q])

            # ---- PE warm-up: keep the HAM activity window busy during the
            # initial DMA wait so real matmuls start at 2.4 GHz.  memset on
            # gpsimd (starts earliest); psum from the apsum ring so G's
            # gpsum tiles are not blocked behind warm-up retirement ----
            wsrc = big.tile([128, 512], bf16, tag="wsrc", name="wsrc")
            nc.gpsimd.memset(wsrc[:], 0.0)
            for wi in range(7):
                wps = apsum.tile([128, 512], fp32, tag="a")
                nc.tensor.matmul(
                    wps[:], wsrc[:, :128], wsrc[:], start=True, stop=True
                )

            # pre-issue Ct quad 1 so the scalar queue's first in-loop
            # trigger is not serialized behind the G cast
            ctt1 = ctp.tile([128, 2048], uint8, tag="ct")
            nc.scalar.dma_start(ctt1[:], ct_d[0, 1])
            ct_pref[(0, 1)] = ctt1

            def fetch_ct(b, j):
                if (b, j) in ct_pref:
                    return ct_pref.pop((b, j))
                t = ctp.tile([128, 2048], uint8, tag="ct")
                nc.scalar.dma_start(t[:], ct_d[b, j])
                return t

            # ---- G = (Wq Wk^T) @ h_loc^T -> [2][128 f, NL]; bk == 0 so no
            # bias.  Block nk is consumed by pass nk; block 1 is emitted
            # inside pass 0's stream once SM piece b has landed ----
            G = [big.tile([128, NL], bf16, tag=f"G{t}", name=f"G{t}") for t in range(2)]

            def make_g(nk, gf):
                def emit():
                    ps = gpsum.tile([128, BW], fp32, tag="g")
                    for ft in range(2):
                        mov = (
                            SMA[:, ft * 768 + D : ft * 768 + D + BW]
                            if nk == 0
                            else SMB[:, ft * 768 : ft * 768 + BW]
                        )
                        nc.tensor.matmul(
                            ps[:],
                            SMA[:, ft * 768 + gf * 128 : ft * 768 + (gf + 1) * 128],
                            mov,
                            start=(ft == 0),
                            stop=(ft == 1),
                        )
                    if gf == 0:
                        nc.scalar.activation(
                            G[gf][:, nk * BW : (nk + 1) * BW], ps[:], Copy
                        )
                    else:
                        nc.vector.tensor_copy(
                            G[gf][:, nk * BW : (nk + 1) * BW], ps[:]
                        )
                return emit

            make_g(0, 0)()
            make_g(0, 1)()
            # two filler matmuls keep the PE busy through the G-cast
            # window so the clock ramp is not reset before A[0]
            for wi in range(2):
                wps = apsum.tile([128, 512], fp32, tag="a")
                nc.tensor.matmul(
                    wps[:], wsrc[:, :128], wsrc[:], start=True, stop=True
                )

            # ---- two column-block passes over all 64 m-chunks ----
            # deferred tensor-engine work dropped into the next pass's stream
            # (G block 1 at c=20/21: SM piece b lands behind hN1 on the queue)
            deferred = {20: make_g(1, 0), 21: make_g(1, 1)}
            for blk in range(NB):
                PT = [
                    ptpsum.tile([128, BW], fp32, tag=f"pt{fh}", name=f"PT{blk}{fh}")
                    for fh in range(2)
                ]
                sts = {}
                for c in range(NCHUNK + 3):
                    if c < NCHUNK:
                        # quad 0 covers chunks 0-3, served by ctf in pass 0
                        if c % 4 == 0 and not (blk == 0 and c == 0):
                            ctt = fetch_ct(blk, c // 4)
                        # A[c] = h-chunk @ G-block  (2 f-half matmuls);
                        # piece 0 is chunk-major (see hT_d comment)
                        aps = apsum.tile([128, BW], fp32, tag="a")
                        for ft in range(2):
                            stat = (
                                hT[c // 8][:, (c % 8) * 256 + ft * 128 : (c % 8) * 256 + (ft + 1) * 128]
                                if c < 16
                                else hT[c // 8][:, ft * 1024 + (c % 8) * 128 : ft * 1024 + (c % 8 + 1) * 128]
                            )
                            nc.tensor.matmul(
                                aps[:],
                                stat,
                                G[ft][:, blk * BW : (blk + 1) * BW],
                                start=(ft == 0),
                                stop=(ft == 1),
                            )
                        # S[c] = Ct * A[c]  (vector engine, PSUM -> SBUF bf16)
                        st = sp.tile([128, BW], bf16, tag="s")
                        csrc = (
                            ctf[:, c * 512 : (c + 1) * 512]
                            if blk == 0 and c < 4
                            else ctt[:, (c % 4) * 512 : (c % 4) * 512 + BW]
                        )
                        nc.vector.tensor_mul(st[:], aps[:], csrc)
                    # P[c-3]: PT[fh] += hN-chunk^T @ S[c-3]  (issued three
                    # chunks late: ~1.8us of slack absorbs the periodic
                    # ~0.85us external hiccup that stalls the vector engine)
                    if c >= 3:
                        cp = c - 3
                        st_prev = sts.pop(cp)
                        for fh in range(2):
                            pstat = (
                                hN[cp // 8][:, (cp % 8) * 256 + fh * 128 : (cp % 8) * 256 + (fh + 1) * 128]
                            )
                            nc.tensor.matmul(
                                PT[fh][:],
                                pstat,
                                st_prev[:],
                                start=(cp == 0),
                                stop=(cp == NCHUNK - 1),
                            )
                    if c < NCHUNK:
                        sts[c] = st
                    if c in deferred:
                        deferred.pop(c)()
                    # pre-trigger the next pass's first Ct quads so they are
                    # not serialized behind the pass-boundary PSUM casts on
                    # the scalar queue
                    if blk == 0 and c == 56:
                        ct_pref[(1, 0)] = fetch_ct(1, 0)
                    if blk == 0 and c == 60:
                        ct_pref[(1, 1)] = fetch_ct(1, 1)

                # ---- pass epilogue: PT -> SBUF bf16 now; agg = W^T @ PT,
                # relu and output DMA deferred into the next pass's stream
                # (for the last pass: emitted immediately, split across
                # scalar/vector for the shortest tail) ----
                last = blk == NB - 1
                PTs = [
                    big.tile([128, BW], bf16, tag=f"PTs{blk}{fh}", name=f"PTs{blk}{fh}")
                    for fh in range(2)
                ]
                if last:
                    nc.scalar.activation(PTs[0][:], PT[0][:], Copy)
                    nc.vector.tensor_copy(PTs[1][:], PT[1][:])
                else:
                    # keep the vector engine free for the next pass's S-mults
                    nc.scalar.activation(PTs[0][:], PT[0][:], Copy)
                    nc.scalar.activation(PTs[1][:], PT[1][:], Copy)
                outS = big.tile([128, NL], fp32, tag=f"out{blk}", name=f"out{blk}")

                def make_agg(blk, dh, PTs=PTs, outS=outS):
                    def emit():
                        agg = gpsum.tile([128, BW], fp32, tag="g")
                        for ft in range(2):
                            nc.tensor.matmul(
                                agg[:],
                                SMB[:, ft * 768 + BW + dh * 128 : ft * 768 + BW + (dh + 1) * 128],
                                PTs[ft][:],
                                start=(ft == 0),
                                stop=(ft == 1),
                            )
                        if dh == 0:
                            nc.scalar.activation(
                                outS[:, dh * BW : (dh + 1) * BW], agg[:], Relu
                            )
                        else:
                            nc.vector.tensor_scalar_max(
                                outS[:, dh * BW : (dh + 1) * BW], agg[:], 0.0
                            )
                        eng = nc.scalar if (blk == NB - 1 and dh == 0) else nc.gpsimd
                        eng.dma_start(
                            out_d[blk * 2 + dh], outS[:, dh * BW : (dh + 1) * BW]
                        )
                    return emit

                if last:
                    make_agg(blk, 0)()
                    make_agg(blk, 1)()
                else:
                    deferred = {2: make_agg(blk, 0), 3: make_agg(blk, 1)}

    nc.compile()
    return nc


def _get_graph():
    if "nc" not in _graph_cache:
        _graph_cache["nc"] = _build_graph()
    return _graph_cache["nc"]


def make_in_maps(h, W, Wq, bq, Wk, bk, senders, receivers):
    h = np.asarray(h, dtype=np.float32)
    W = np.asarray(W, dtype=np.float32)
    Wq = np.asarray(Wq, dtype=np.float32)
    Wk = np.asarray(Wk, dtype=np.float32)
    bq = np.asarray(bq, dtype=np.float32)
    bk = np.asarray(bk, dtype=np.float32)
    s = np.asarray(senders).astype(np.int64)
    r = np.asarray(receivers).astype(np.int64)

    # bq == bk == 0 (module spec fills both with zeros) lets
    # A = h @ ((Wq Wk^T) h_loc^T) stand in exactly for q @ k^T.
    assert not np.any(bq), "kernel fast path assumes bq == 0"
    assert not np.any(bk), "kernel fast path assumes bk == 0"

    hT = np.ascontiguousarray(h.T).astype(BF16)  # [256, 8192]
    # piece k = m-chunks 8k..8k+7: cols [f-half0 1024 | f-half1 1024];
    # piece 0 is chunk-major (cols c*256 + ft*128) so its halves split at
    # chunk 4 for the early DMA
    hT8 = (
        hT.reshape(2, 128, 8, 1024)
        .transpose(2, 1, 0, 3)
        .reshape(8, 128, 2048)
    )
    hT8 = np.ascontiguousarray(hT8)
    for k in range(2):
        hT8[k] = (
            hT8[k].reshape(128, 2, 8, 128).transpose(0, 2, 1, 3).reshape(128, 2048)
        )
    hN8 = (
        h.astype(BF16)
        .reshape(8, 8, 128, D)
        .transpose(0, 2, 1, 3)
        .reshape(8, 128, 8 * D)
    )
    # folded attention weight product (parameter preprocessing):
    # G = (Wq Wk^T) h_loc^T  ==  q-free form of q @ k_c^T
    WqkT = (Wk @ Wq.T).astype(BF16).reshape(2, 128, D)
    Wb = W.astype(BF16).reshape(2, 128, D)

    in_maps = []
    for c in range(N_CORES):
        lo = c * NL
        m = (r >= lo) & (r < lo + NL)
        idx = s[m] * NL + (r[m] - lo)
        Ct = np.bincount(idx, minlength=N_NODES * NL)
        assert Ct.max() < 128
        Ct = Ct.astype(np.uint8).reshape(NCHUNK, 128, NL)
        # [blk][quad j][128][chunk 4j+i at cols i*512]
        Ctb = np.empty((NB, NCHUNK // 4, 128, 2048), np.uint8)
        for b in range(NB):
            blkslice = Ct[:, :, b * BW : (b + 1) * BW]  # [64,128,512]
            for i in range(4):
                Ctb[b, :, :, i * 512 : (i + 1) * 512] = blkslice[i::4]
        hTloc = hT.reshape(2, 128, N_NODES)[:, :, lo : lo + NL]
        # SMA = per f-half [WqkT | hTloc blk0]; SMB = [hTloc blk1 | W]
        SMA = np.concatenate(
            [np.concatenate([WqkT[t], hTloc[t][:, 0:BW]], axis=1) for t in range(2)],
            axis=1,
        )
        SMB = np.concatenate(
            [np.concatenate([hTloc[t][:, BW:NL], Wb[t]], axis=1) for t in range(2)],
            axis=1,
        )
        # unpacked block-0 counts for chunks 0-3 at cols c*512
        ctf = np.ascontiguousarray(
            np.concatenate([Ct[cc, :, 0:BW] for cc in range(4)], axis=1)
        )
        in_maps.append(
            {
                "hT": hT8,
                "hN": hN8,
                "SMA": np.ascontiguousarray(SMA),
                "SMB": np.ascontiguousarray(SMB),
                "Ct": Ctb,
                "ctf": ctf,
            }
        )
    return in_maps


def assemble_output(results):
    out = np.empty((N_NODES, D), np.float32)
    for c in range(N_CORES):
        res = np.asarray(results[c]["out"]).reshape(NB, 2, 128, BW)
        for b in range(NB):
            # res[b][dh, p, n] -> out[lo + b*512 + n, dh*128 + p]
            out[c * NL + b * BW : c * NL + (b + 1) * BW] = (
                res[b].transpose(2, 0, 1).reshape(BW, D)
            )
    return out


def kernel(h, W, Wq, bq, Wk, bk, senders, receivers):
    from concourse.bass_utils import run_bass_kernel_spmd

    in_maps = make_in_maps(h, W, Wq, bq, Wk, bk, senders, receivers)
    nc = _get_graph()
    res = run_bass_kernel_spmd(nc, in_maps, list(range(N_CORES))).results
    return assemble_output(res)
